# revision 2
# baseline (speedup 1.0000x reference)
"""Trainium2 Bass kernel for nn_MeshHandler (GNN message passing).

weights[n] = segment_sum over (e,k) of MLP(points[adjacency[e]])[k]

Strategy (per sharding hint): shard elements E=4M across 8 cores (500K each),
points replicated in HBM. Per core, a hardware loop processes 512 elements per
iteration:
  - load adjacency tile [128, 12] (int32, host-preformatted)
  - 12 indirect DMA gathers (128 offsets each) pull point pairs into SBUF
  - PE transposes -> features-on-partitions [6, 512]
  - 4-layer MLP (matmuls on PE + sigmoid on ACT)
  - PE transposes back -> per-pair weights [128, 12]
  - 12 indirect DMA scatters with CCE-add into 4 rotating HBM accumulators
    (rotation avoids concurrent read-modify-write races between in-flight
    scatter DMAs; within-DMA duplicate indices are statistically negligible)
  - final dense reduction of the 4 accumulators on device
Host side: shard/pad/format adjacency (int64->int32), sum the 8 per-core
partial outputs.
"""
import numpy as np

import concourse.bacc as bacc
import concourse.bass as bass
import concourse.mybir as mybir
import concourse.tile as tile
import concourse.bass_utils as _bu
from concourse.bass import ds
from concourse.masks import make_identity
from concourse.bass_utils import run_bass_kernel_spmd

f32 = mybir.dt.float32
i32 = mybir.dt.int32

N_POINTS = 2_000_000
N_ELEMS = 4_000_000
NCORES = 8
EC = N_ELEMS // NCORES          # 500_000 elements per core
BODY = 512                      # elements per loop iteration
NB = (EC + BODY - 1) // BODY    # 977 iterations
ECP = NB * BODY                 # 500_224 padded elements per core
NPAD = N_POINTS + 128           # padded point/bin count, = 128 * 15626
FPP = NPAD // 128               # 15626
PAD_IDX = N_POINTS              # trash bin / zero point row

# ---- walrus patch: enable dynamic-offset DGE levels for indirect DMA ----
_orig_run_command = _bu.run_command
_DGE_ARG = ("--dge-levels=io,spill_reload,scalar_dynamic_offset,"
            "vector_dynamic_offsets,dynamic_size,dst_reduce")


def _run_command_patched(argv, **kwargs):
    if argv and "walrus_driver" in str(argv[0]) and not any(
            "--dge-levels" in str(a) for a in argv):
        argv = argv[:1] + [_DGE_ARG] + argv[1:]
    return _orig_run_command(argv, **kwargs)


if _bu.run_command is not _run_command_patched:
    _bu.run_command = _run_command_patched

_NC_CACHE = {}


def _build():
    if "nc" in _NC_CACHE:
        return _NC_CACHE["nc"]
    nc = bacc.Bacc("TRN2", target_bir_lowering=False, debug=False,
                   num_devices=NCORES)
    points_d = nc.dram_tensor("points", [NPAD, 2], f32, kind="ExternalInput")
    adj_d = nc.dram_tensor("adj", [NB * 128, 12], i32, kind="ExternalInput")
    w_d = [nc.dram_tensor(f"w{i}", s, f32, kind="ExternalInput")
           for i, s in enumerate([[6, 8], [8, 8], [8, 8], [8, 3]], start=1)]
    b_d = [nc.dram_tensor(f"b{i}", s, f32, kind="ExternalInput")
           for i, s in enumerate([[8, 1], [8, 1], [8, 1], [3, 1]], start=1)]
    out_d = nc.dram_tensor("out", [NPAD, 1], f32, kind="ExternalOutput")
    acc_d = [nc.dram_tensor(f"acc{r}", [NPAD, 1], f32, kind="Internal")
             for r in range(4)]

    with tile.TileContext(nc) as tc:
        with tc.tile_pool(name="const", bufs=1) as cpool, \
             tc.tile_pool(name="sb", bufs=2) as pool, \
             tc.tile_pool(name="red", bufs=2) as rpool, \
             tc.tile_pool(name="ps", bufs=2, space="PSUM") as psum:
            ident = cpool.tile([128, 128], f32)
            make_identity(nc, ident[:])
            w_t = []
            b_t = []
            for i in range(4):
                wt = cpool.tile(list(w_d[i].shape), f32, tag=f"w{i}")
                nc.sync.dma_start(wt[:], w_d[i][:])
                w_t.append(wt)
                bt = cpool.tile(list(b_d[i].shape), f32, tag=f"b{i}")
                nc.sync.dma_start(bt[:], b_d[i][:])
                b_t.append(bt)

            # zero the rotating accumulators
            zt = cpool.tile([128, FPP], f32)
            nc.vector.memset(zt[:], 0.0)
            for r in range(4):
                nc.sync.dma_start(
                    acc_d[r][:].rearrange("(a b) o -> a (b o)", a=128), zt[:])

            with tc.For_i(0, NB * 128, 128) as trow:
                idx_t = pool.tile([128, 12], i32)
                nc.sync.dma_start(idx_t[:], adj_d[ds(trow, 128), :])
                G = pool.tile([128, 24], f32)
                for j in range(4):
                    for k in range(3):
                        c = 3 * j + k
                        nc.gpsimd.indirect_dma_start(
                            out=G[:, 6 * j + 2 * k: 6 * j + 2 * k + 2],
                            out_offset=None,
                            in_=points_d[:],
                            in_offset=bass.IndirectOffsetOnAxis(
                                ap=idx_t[:, c:c + 1], axis=0),
                        )
                # transpose 4x [128, 6] -> [6, 512] (features on partitions)
                x_ps = psum.tile([6, BODY], f32)
                for j in range(4):
                    nc.tensor.matmul(
                        out=x_ps[:, 128 * j:128 * (j + 1)],
                        lhsT=G[:, 6 * j:6 * (j + 1)],
                        rhs=ident[:],
                        start=True, stop=True)
                x_t = pool.tile([6, BODY], f32)
                nc.vector.tensor_copy(x_t[:], x_ps[:])

                h = x_t
                for layer in range(4):
                    m = w_t[layer].shape[1]  # 8, 8, 8, 3
                    h_ps = psum.tile([8, BODY], f32, tag="h")
                    nc.tensor.matmul(out=h_ps[:m], lhsT=w_t[layer][:],
                                     rhs=h[:], start=True, stop=True)
                    s_t = pool.tile([m, BODY], f32, tag=f"s{layer}")
                    nc.scalar.activation(
                        s_t[:], h_ps[:m],
                        mybir.ActivationFunctionType.Sigmoid,
                        bias=b_t[layer][:])
                    h = s_t
                # back-transpose [3, 512] -> [128, 12] (pair weights)
                v_ps = psum.tile([128, 12], f32)
                for j in range(4):
                    nc.tensor.matmul(
                        out=v_ps[:, 3 * j:3 * (j + 1)],
                        lhsT=h[:, 128 * j:128 * (j + 1)],
                        rhs=ident[:3, :3],
                        start=True, stop=True)
                v_t = pool.tile([128, 12], f32)
                nc.vector.tensor_copy(v_t[:], v_ps[:])
                for j in range(4):
                    for k in range(3):
                        c = 3 * j + k
                        nc.gpsimd.indirect_dma_start(
                            out=acc_d[j][:],
                            out_offset=bass.IndirectOffsetOnAxis(
                                ap=idx_t[:, c:c + 1], axis=0),
                            in_=v_t[:, c:c + 1],
                            in_offset=None,
                            compute_op=mybir.AluOpType.add,
                        )

            # dense reduce: out = acc0 + acc1 + acc2 + acc3
            half = FPP // 2  # 7813
            for c0 in (0, half):
                t0 = rpool.tile([128, half], f32, tag="r0")
                nc.sync.dma_start(
                    t0[:],
                    acc_d[0][:].rearrange("(a b) o -> a (b o)", a=128)[:, c0:c0 + half])
                for r in range(1, 4):
                    tr = rpool.tile([128, half], f32, tag="r1")
                    nc.sync.dma_start(
                        tr[:],
                        acc_d[r][:].rearrange("(a b) o -> a (b o)", a=128)[:, c0:c0 + half])
                    nc.vector.tensor_tensor(out=t0[:], in0=t0[:], in1=tr[:],
                                            op=mybir.AluOpType.add)
                nc.sync.dma_start(
                    out_d[:].rearrange("(a b) o -> a (b o)", a=128)[:, c0:c0 + half],
                    t0[:])
    nc.compile()
    _NC_CACHE["nc"] = nc
    return nc


def _prep_inputs(points, adjacency, W1, b1, W2, b2, W3, b3, W4, b4):
    points_pad = np.zeros((NPAD, 2), np.float32)
    points_pad[:N_POINTS] = np.asarray(points, np.float32)
    adj = np.asarray(adjacency).astype(np.int32)  # values < 2M fit in int32
    common = dict(
        points=points_pad,
        w1=np.asarray(W1, np.float32), w2=np.asarray(W2, np.float32),
        w3=np.asarray(W3, np.float32), w4=np.asarray(W4, np.float32),
        b1=np.asarray(b1, np.float32).reshape(8, 1),
        b2=np.asarray(b2, np.float32).reshape(8, 1),
        b3=np.asarray(b3, np.float32).reshape(8, 1),
        b4=np.asarray(b4, np.float32).reshape(3, 1),
    )
    in_maps = []
    for c in range(NCORES):
        sl = adj[c * EC:(c + 1) * EC]
        slp = np.full((ECP, 3), PAD_IDX, np.int32)
        slp[:EC] = sl
        # element e = t*512 + j*128 + q  ->  adj_tile[t*128+q, 3*j+k]
        a = slp.reshape(NB, 4, 128, 3).transpose(0, 2, 1, 3).reshape(NB * 128, 12)
        in_maps.append(dict(common, adj=np.ascontiguousarray(a)))
    return in_maps


def kernel(points, adjacency, W1, b1, W2, b2, W3, b3, W4, b4,
           _collect_perf=None):
    nc = _build()
    in_maps = _prep_inputs(points, adjacency, W1, b1, W2, b2, W3, b3, W4, b4)
    kwargs = {}
    if _collect_perf is not None:
        kwargs["trace"] = True
    res = run_bass_kernel_spmd(nc, in_maps, core_ids=list(range(NCORES)),
                               **kwargs)
    if _collect_perf is not None and res.exec_time_ns is not None:
        _collect_perf["exec_time_ns"] = res.exec_time_ns
        _collect_perf["trace"] = res.instructions_and_trace
    total = np.zeros(N_POINTS, np.float64)
    for r in res.results:
        total += r["out"][:N_POINTS, 0].astype(np.float64)
    return total.astype(np.float32)
